# revision 49
# baseline (speedup 1.0000x reference)
"""Trainium2 Bass kernel for segment-reduce classifier (v3).

Reference computation:
    local = relu(x @ Wloc.T)            # [L, 128]
    feats = local.reshape(-1, 30, 128).mean(1)   # [L/30, 128]
    out   = feats @ W.T                 # [L/30, 10]

The kernel is PSUM-drain bound: every local element (fp32 in PSUM) must be
relu'd + copied to SBUF by ScalarE (1.2 GHz) or VectorE (0.96 GHz), each
limited to 1 elem/lane/cycle from PSUM (GPSIMD and DMA have no PSUM port).
Combined floor ~70us/core for 150000 elems/lane; everything else is
scheduled to stay off that critical path.

Design (per core, data-parallel rows; fp16 end to end, median rel err
~1.5e-4):
  - x shard host-transposed and host-PERMUTED so each 510-col chunk
    (17 segments x 30 offsets) is already j-major; fp16 tiles of up to 16
    chunks stream through a 5-deep SBUF pool on both HWDGE(sync) and
    SWDGE(gpsimd) DMA rings, small tiles first for a fast ramp.
  - mm1: per chunk, two concurrent K=64 row-group matmuls (partitions 0-63 =
    first half-stream transposed, 64-127 = second) fill one 2-bank PSUM tile
    [128, 1024] fp32; psp bufs=3 rotation (6 banks).
  - drain: ONE relu instruction per tile, FD=1020, contiguous reads/writes
    (the host permutation removed the j-scatter), greedily assigned to
    ScalarE/VectorE by measured per-op cost to balance their busy time.
  - mm2 (pool+classifier): per group of chunks, 30 accumulating j-matmuls
    (rhs = [128, 2, nch, 17] slices of the relu'd buffer), C=10 packed 4x
    into PE column strips; acc [128, <=510] fp32 in 1 of 2 dedicated banks.
    The 8 "k-slices" (one j per strip) are spread one-per-chunk across the
    NEXT group and the first chunks of the one after, each emitted BEFORE
    that chunk's encoder pair, so the cold (HAM-throttled) PE always keeps
    slack over the drain cadence and no drain queue blocks on classifier
    work. Host sums the 4 strips.
  - acc is drained per group into a small staging tile and leaves via one
    full-partition DMA per group, so there is no output tail.
"""

import numpy as np
import ml_dtypes

import concourse.bacc as bacc
import concourse.bass as bass
import concourse.tile as tile
from concourse import mybir
from concourse.bass_utils import run_bass_kernel_spmd

# Problem constants (hardcoded per harness contract)
L, D_IN, D_ENC, C, J = 1200000, 64, 128, 10, 30
N_CORES = 8
R = L // N_CORES            # rows per core = 150000
HALF = R // 2               # 75000 rows per half-stream
SEG_H = HALF // J           # 2500 real segments per half
CH = 510                    # chunk cols = 17 segments * 30
GSEG = CH // J              # 17 segments per chunk per half
NCHUNK = 148                # ceil(75000/510) -> padded to 148*510
COLS = NCHUNK * CH          # 75480 padded cols per half
GROUPS = [15] * 9 + [9, 4]  # chunks per mm2 accumulation group (sum=148)
SLOTS_H = NCHUNK * GSEG     # 2516 segment slots per half (incl. 16 bogus)
OUT_COLS = 2 * GSEG * sum(GROUPS)  # 5032 staged output cols
# j-subsets for the 4 PE column-group strips of the classifier matmul
J_SETS = [list(range(0, 8)), list(range(8, 16)),
          list(range(16, 23)), list(range(23, 30))]

_CACHE = {}

# modeled per-drain-instruction cost (ns) for greedy engine balancing
def _act_cost(fd):
    return (311.0 + fd) / 1.2

def _dve_cost(fd):
    return (62.0 + fd) / 0.96


def _build_kernel():
    nc = bacc.Bacc("TRN2", target_bir_lowering=False, debug=False,
                   num_devices=N_CORES)
    f32, f16, f8 = mybir.dt.float32, mybir.dt.float16, mybir.dt.float8e4

    XDT = mybir.dt.float16
    xt_d = nc.dram_tensor("xt", [128, COLS], XDT, kind="ExternalInput")
    w1_d = nc.dram_tensor("w1", [128, D_ENC], f16, kind="ExternalInput")
    w2_d = nc.dram_tensor("w2", [128, C], f16, kind="ExternalInput")
    out_d = nc.dram_tensor("out", [128, OUT_COLS], f32, kind="ExternalOutput")

    with tile.TileContext(nc) as tc:
        with (
            tc.tile_pool(name="consts", bufs=1) as consts,
            tc.tile_pool(name="xin", bufs=5) as xin,
            tc.tile_pool(name="rlp", bufs=3) as rlp,
            tc.tile_pool(name="outp", bufs=2) as outp,
            tc.tile_pool(name="psp", bufs=3, space="PSUM") as psp,
            tc.tile_pool(name="accp", bufs=2, space="PSUM") as accp,
        ):
            w1 = consts.tile([128, D_ENC], f16)
            nc.sync.dma_start(w1[:], w1_d[:])
            w2 = consts.tile([128, C], f16)
            nc.sync.dma_start(w2[:], w2_d[:])

            # streamed input tiles, small first ones for fast start
            TB = [0, 4, 12, 24, 40, 56, 72, 88, 104, 120, 136, 148]
            xtiles = []
            for i, (a, b) in enumerate(zip(TB, TB[1:])):
                xtl = xin.tile([128, 16 * CH], XDT, tag="xt", name="xt")
                eng = nc.sync if (i < 4 or i % 2 == 0) else nc.gpsimd
                eng.dma_start(xtl[:, 0:(b - a) * CH],
                              xt_d[:, a * CH:b * CH])
                xtiles.append((a, xtl))

            t_act = 0.0
            t_dve = 0.0

            def drain(rout, pin, fd):
                nonlocal t_act, t_dve
                ca, cd = _act_cost(fd), _dve_cost(fd)
                if t_act + ca <= t_dve + cd:
                    t_act += ca
                    nc.scalar.activation(rout, pin,
                                         mybir.ActivationFunctionType.Relu)
                else:
                    t_dve += cd
                    nc.vector.tensor_scalar_max(rout, pin, 0.0)

            # group state
            gidx = 0
            gstart = 0            # first chunk of current group
            rl = None
            rlv = None
            ocol = 0

            def mm2_kslice(pend, k):
                p_rl, nch, acc = pend["rl"], pend["nch"], pend["acc"]
                n = 2 * nch * GSEG
                rhs_all = p_rl.rearrange("p (h q) -> p h q", h=2)[
                    :, :, 0:nch * CH].rearrange(
                    "p h (c j g) -> p h c j g", c=nch, j=J)
                for s in range(4):
                    if k >= len(J_SETS[s]):
                        continue
                    j = J_SETS[s][k]
                    nc.tensor.matmul(acc[32 * s:32 * s + C, 0:n],
                                     w2[:], rhs_all[:, :, :, j, :],
                                     start=(k == 0),
                                     stop=(k == len(J_SETS[s]) - 1),
                                     tile_position=(0, 32 * s))

            def mm2_finish(pend):
                # drain accumulator -> staging, then compact strip DMAs out
                nch, p_ocol, acc = pend["nch"], pend["ocol"], pend["acc"]
                n = 2 * nch * GSEG
                nonlocal t_act, t_dve
                out_sb = outp.tile([128, 512], f32, tag="osb", name="osb")
                ca, cd = _act_cost(n), _dve_cost(n)
                if t_act + ca <= t_dve + cd:
                    t_act += ca
                    nc.scalar.copy(out_sb[:, 0:n], acc[:, 0:n])
                else:
                    t_dve += cd
                    nc.vector.tensor_copy(out_sb[:, 0:n], acc[:, 0:n])
                nc.sync.dma_start(out_d[:, p_ocol:p_ocol + n],
                                  out_sb[:, 0:n])

            # k-slice schedule: group g's 8 classifier slices are spread
            # over group g+1 (k0-k6) and the first chunks of group g+2 (k7 at
            # lc=0, accumulator copy at lc=2), one slice per chunk and always
            # BEFORE the chunk's encoder pair, so the cold PE keeps slack over
            # the drain cadence and no drain queue blocks on an unfinished
            # classifier matmul. pA = group that just ended, pB = one before.
            SLOTSETS = {15: (1, 3, 5, 7, 9, 11, 13), 9: (1, 2, 3, 4, 5, 6, 7),
                        4: (1, 2, 3)}
            pA = None
            pB = None
            ti = 0

            def pend_kslice(pend):
                if pend["next_k"] == 0:
                    pend["acc"] = accp.tile([128, 512], f32, tag="acc",
                                            name="acc")
                mm2_kslice(pend, pend["next_k"])
                pend["next_k"] += 1

            for c in range(NCHUNK):
                if c == gstart:
                    rl = rlp.tile([128, 2 * 15 * CH], f16, tag="rl")
                    rlv = rl.rearrange("p (h q) -> p h q", h=2)
                lc = c - gstart
                k7s = 2 if GROUPS[gidx] == 15 else 0
                if pB is not None:
                    if lc == k7s and pB["next_k"] == 7:
                        pend_kslice(pB)
                    elif lc == k7s + 2:
                        mm2_finish(pB)
                        pB = None
                if (pA is not None and lc in SLOTSETS[GROUPS[gidx]]
                        and pA["next_k"] < 7):
                    pend_kslice(pA)
                while ti + 1 < len(TB) and c >= TB[ti + 1]:
                    ti += 1
                xa, xtl = xtiles[ti]
                o0 = (c - xa) * CH
                pp = psp.tile([128, 1024], f32, tag="pp")
                nc.tensor.matmul(pp[:, 0:CH], w1[0:64, :],
                                 xtl[0:64, o0:o0 + CH])
                nc.tensor.matmul(pp[:, 512:512 + CH], w1[64:128, :],
                                 xtl[64:128, o0:o0 + CH])
                pin = pp.rearrange("p (h q) -> p h q", h=2)[:, :, 0:CH]
                rout = rlv[:, :, lc * CH:(lc + 1) * CH]
                drain(rout, pin, 2 * CH)
                if lc == GROUPS[gidx] - 1:
                    assert pB is None
                    pB = pA
                    pA = {"rl": rl, "nch": GROUPS[gidx], "ocol": ocol,
                          "acc": None, "next_k": 0}
                    ocol += 2 * GROUPS[gidx] * GSEG
                    gstart = c + 1
                    gidx += 1
            # tail: finish whatever classifier work remains, oldest first
            for pend in (pB, pA):
                if pend is None:
                    continue
                while pend["next_k"] < 8:
                    pend_kslice(pend)
                mm2_finish(pend)

    nc.compile()
    return nc


def _pack_inputs(x, Wloc, W):
    x = np.asarray(x, dtype=np.float32)
    # [core, half, row, d] -> pad rows to 148*510 -> j-major per 510-chunk
    xp = x.reshape(N_CORES, 2, HALF, D_IN)
    pad = np.zeros((N_CORES, 2, COLS - HALF, D_IN), dtype=np.float32)
    xp = np.concatenate([xp, pad], axis=2)
    # within each chunk: source row = g*30 + j  ->  column j*17 + g
    xp = xp.reshape(N_CORES, 2, NCHUNK, GSEG, J, D_IN)
    xp = xp.transpose(0, 1, 2, 4, 3, 5)          # [., c, j, g, d]
    xp = xp.reshape(N_CORES, 2, COLS, D_IN)
    xp = xp.transpose(0, 1, 3, 2)                # [core, half, d, col]
    xp8 = np.ascontiguousarray(xp, dtype=np.float16).reshape(
        N_CORES, 128, COLS)

    w1 = np.ascontiguousarray(
        np.concatenate([Wloc.T, Wloc.T], axis=0), dtype=np.float16)
    w2 = np.ascontiguousarray((W / float(J)).T, dtype=np.float16)
    return xp8, w1, w2


def kernel(x: np.ndarray, Wloc: np.ndarray, W: np.ndarray) -> np.ndarray:
    if "nc" not in _CACHE:
        _CACHE["nc"] = _build_kernel()
    nc = _CACHE["nc"]

    xp8, w1, w2 = _pack_inputs(x, Wloc, W)
    in_maps = [{"xt": xp8[c], "w1": w1, "w2": w2} for c in range(N_CORES)]
    res = run_bass_kernel_spmd(nc, in_maps, core_ids=list(range(N_CORES)))
    _CACHE["exec_time_ns"] = res.exec_time_ns
    _CACHE["trace"] = res.instructions_and_trace

    out = np.empty((L // J, C), dtype=np.float32)
    for core in range(N_CORES):
        od = res.results[core]["out"]            # [128, OUT_COLS]
        oc = od[0:10] + od[32:42] + od[64:74] + od[96:106]  # [10, 5032]
        seg_vals = np.empty((2, SLOTS_H, C), dtype=np.float32)
        off = 0
        cstart = 0
        for nch in GROUPS:
            n = 2 * nch * GSEG
            blk = oc[:, off:off + n].reshape(C, 2, nch * GSEG)
            s0 = cstart * GSEG
            seg_vals[0, s0:s0 + nch * GSEG] = blk[:, 0].T
            seg_vals[1, s0:s0 + nch * GSEG] = blk[:, 1].T
            off += n
            cstart += nch
        base = core * (R // J)
        out[base:base + SEG_H] = seg_vals[0, :SEG_H]
        out[base + SEG_H:base + 2 * SEG_H] = seg_vals[1, :SEG_H]
    return out


# revision 51
# speedup vs baseline: 1.0709x; 1.0709x over previous
"""Trainium2 Bass kernel for segment-reduce classifier (v3).

Reference computation:
    local = relu(x @ Wloc.T)            # [L, 128]
    feats = local.reshape(-1, 30, 128).mean(1)   # [L/30, 128]
    out   = feats @ W.T                 # [L/30, 10]

The kernel is PSUM-drain bound: every local element (fp32 in PSUM) must be
relu'd + copied to SBUF by ScalarE (1.2 GHz) or VectorE (0.96 GHz), each
limited to 1 elem/lane/cycle from PSUM (GPSIMD and DMA have no PSUM port).
Combined floor ~70us/core for 150000 elems/lane; everything else is
scheduled to stay off that critical path.

Design (per core, data-parallel rows; fp16 end to end, median rel err
~1.5e-4):
  - x shard host-transposed and host-PERMUTED so each 510-col chunk
    (17 segments x 30 offsets) is already j-major; fp16 tiles of up to 16
    chunks stream through a 5-deep SBUF pool on both HWDGE(sync) and
    SWDGE(gpsimd) DMA rings, small tiles first for a fast ramp.
  - mm1: per chunk, two concurrent K=64 row-group matmuls (partitions 0-63 =
    first half-stream transposed, 64-127 = second) fill one 2-bank PSUM tile
    [128, 1024] fp32; psp bufs=3 rotation (6 banks).
  - drain: ONE relu instruction per tile, FD=1020, contiguous reads/writes
    (the host permutation removed the j-scatter), greedily assigned to
    ScalarE/VectorE by measured per-op cost to balance their busy time.
  - mm2 (pool+classifier): per group of chunks, 30 accumulating j-matmuls
    (rhs = [128, 2, nch, 17] slices of the relu'd buffer), C=10 packed 4x
    into PE column strips; acc [128, <=510] fp32 in 1 of 2 dedicated banks.
    The 8 "k-slices" (one j per strip) are spread one-per-chunk across the
    NEXT group and the first chunks of the one after, each emitted BEFORE
    that chunk's encoder pair, so the cold (HAM-throttled) PE always keeps
    slack over the drain cadence and no drain queue blocks on classifier
    work. Host sums the 4 strips.
  - acc is drained per group into a small staging tile and leaves via one
    full-partition DMA per group, so there is no output tail.
"""

import numpy as np

import concourse.bacc as bacc
import concourse.bass as bass
import concourse.tile as tile
from concourse import mybir
from concourse.bass_utils import run_bass_kernel_spmd

# Problem constants (hardcoded per harness contract)
L, D_IN, D_ENC, C, J = 1200000, 64, 128, 10, 30
N_CORES = 8
R = L // N_CORES            # rows per core = 150000
HALF = R // 2               # 75000 rows per half-stream
SEG_H = HALF // J           # 2500 real segments per half
CH = 510                    # chunk cols = 17 segments * 30
GSEG = CH // J              # 17 segments per chunk per half
NCHUNK = 148                # ceil(75000/510) -> padded to 148*510
COLS = NCHUNK * CH          # 75480 padded cols per half
GROUPS = [15] * 9 + [9, 4]  # chunks per mm2 accumulation group (sum=148)
SLOTS_H = NCHUNK * GSEG     # 2516 segment slots per half (incl. 16 bogus)
OUT_COLS = 2 * GSEG * sum(GROUPS)  # 5032 staged output cols
# j-subsets for the 4 PE column-group strips of the classifier matmul
J_SETS = [list(range(0, 8)), list(range(8, 16)),
          list(range(16, 23)), list(range(23, 30))]

_CACHE = {}

# modeled per-drain-instruction cost (ns) for greedy engine balancing
def _act_cost(fd):
    return (311.0 + fd) / 1.2

def _dve_cost(fd):
    return (62.0 + fd) / 0.96


def _build_kernel():
    nc = bacc.Bacc("TRN2", target_bir_lowering=False, debug=False,
                   num_devices=N_CORES)
    f32, f16, f8 = mybir.dt.float32, mybir.dt.float16, mybir.dt.float8e4

    XDT = mybir.dt.float16
    xt_d = nc.dram_tensor("xt", [128, COLS], XDT, kind="ExternalInput")
    w1_d = nc.dram_tensor("w1", [128, D_ENC], f16, kind="ExternalInput")
    w2_d = nc.dram_tensor("w2", [128, C], f16, kind="ExternalInput")
    out_d = nc.dram_tensor("out", [128, OUT_COLS], f32, kind="ExternalOutput")

    with tile.TileContext(nc) as tc:
        with (
            tc.tile_pool(name="consts", bufs=1) as consts,
            tc.tile_pool(name="xin", bufs=5) as xin,
            tc.tile_pool(name="rlp", bufs=3) as rlp,
            tc.tile_pool(name="outp", bufs=2) as outp,
            tc.tile_pool(name="psp", bufs=3, space="PSUM") as psp,
            tc.tile_pool(name="accp", bufs=2, space="PSUM") as accp,
        ):
            w1 = consts.tile([128, D_ENC], f16)
            nc.sync.dma_start(w1[:], w1_d[:])
            w2 = consts.tile([128, C], f16)
            nc.sync.dma_start(w2[:], w2_d[:])

            # streamed input tiles, small first ones for fast start
            TB = [0, 4, 9, 15, 22, 30, 40, 52, 68, 84, 100, 116, 132, 148]
            xtiles = []
            for i, (a, b) in enumerate(zip(TB, TB[1:])):
                xtl = xin.tile([128, 16 * CH], XDT, tag="xt", name="xt")
                eng = nc.sync if (i < 4 or i % 2 == 0) else nc.gpsimd
                eng.dma_start(xtl[:, 0:(b - a) * CH],
                              xt_d[:, a * CH:b * CH])
                xtiles.append((a, xtl))

            t_act = 0.0
            t_dve = 0.0

            def drain(rout, pin, fd):
                nonlocal t_act, t_dve
                ca, cd = _act_cost(fd), _dve_cost(fd)
                if t_act + ca <= t_dve + cd:
                    t_act += ca
                    nc.scalar.activation(rout, pin,
                                         mybir.ActivationFunctionType.Relu)
                else:
                    t_dve += cd
                    nc.vector.tensor_scalar_max(rout, pin, 0.0)

            # group state
            gidx = 0
            gstart = 0            # first chunk of current group
            rl = None
            rlv = None
            ocol = 0

            def mm2_kslice(pend, k):
                p_rl, nch, acc = pend["rl"], pend["nch"], pend["acc"]
                n = 2 * nch * GSEG
                rhs_all = p_rl.rearrange("p (h q) -> p h q", h=2)[
                    :, :, 0:nch * CH].rearrange(
                    "p h (c j g) -> p h c j g", c=nch, j=J)
                for s in range(4):
                    if k >= len(J_SETS[s]):
                        continue
                    j = J_SETS[s][k]
                    nc.tensor.matmul(acc[32 * s:32 * s + C, 0:n],
                                     w2[:], rhs_all[:, :, :, j, :],
                                     start=(k == 0),
                                     stop=(k == len(J_SETS[s]) - 1),
                                     tile_position=(0, 32 * s))

            def mm2_finish(pend):
                # drain accumulator -> staging, then compact strip DMAs out
                nch, p_ocol, acc = pend["nch"], pend["ocol"], pend["acc"]
                n = 2 * nch * GSEG
                nonlocal t_act, t_dve
                out_sb = outp.tile([128, 512], f32, tag="osb", name="osb")
                ca, cd = _act_cost(n), _dve_cost(n)
                if t_act + ca <= t_dve + cd:
                    t_act += ca
                    nc.scalar.copy(out_sb[:, 0:n], acc[:, 0:n])
                else:
                    t_dve += cd
                    nc.vector.tensor_copy(out_sb[:, 0:n], acc[:, 0:n])
                nc.sync.dma_start(out_d[:, p_ocol:p_ocol + n],
                                  out_sb[:, 0:n])

            # k-slice schedule: group g's 8 classifier slices are spread
            # over group g+1 (k0-k6) and the first chunks of group g+2 (k7 at
            # lc=0, accumulator copy at lc=2), one slice per chunk and always
            # BEFORE the chunk's encoder pair, so the cold PE keeps slack over
            # the drain cadence and no drain queue blocks on an unfinished
            # classifier matmul. pA = group that just ended, pB = one before.
            SLOTSETS = {15: (1, 3, 5, 7, 9, 11, 13), 9: (1, 2, 3, 4, 5, 6, 7),
                        4: (1, 2, 3)}
            pA = None
            pB = None
            ti = 0

            def pend_kslice(pend):
                if pend["next_k"] == 0:
                    pend["acc"] = accp.tile([128, 512], f32, tag="acc",
                                            name="acc")
                mm2_kslice(pend, pend["next_k"])
                pend["next_k"] += 1

            for c in range(NCHUNK):
                if c == gstart:
                    rl = rlp.tile([128, 2 * 15 * CH], f16, tag="rl")
                    rlv = rl.rearrange("p (h q) -> p h q", h=2)
                lc = c - gstart
                k7s = 2 if GROUPS[gidx] == 15 else 0
                if pB is not None:
                    if lc == k7s and pB["next_k"] == 7:
                        pend_kslice(pB)
                    elif lc == k7s + 2:
                        mm2_finish(pB)
                        pB = None
                if (pA is not None and lc in SLOTSETS[GROUPS[gidx]]
                        and pA["next_k"] < 7):
                    pend_kslice(pA)
                while ti + 1 < len(TB) and c >= TB[ti + 1]:
                    ti += 1
                xa, xtl = xtiles[ti]
                o0 = (c - xa) * CH
                pp = psp.tile([128, 1024], f32, tag="pp")
                nc.tensor.matmul(pp[:, 0:CH], w1[0:64, :],
                                 xtl[0:64, o0:o0 + CH])
                nc.tensor.matmul(pp[:, 512:512 + CH], w1[64:128, :],
                                 xtl[64:128, o0:o0 + CH])
                pin = pp.rearrange("p (h q) -> p h q", h=2)[:, :, 0:CH]
                rout = rlv[:, :, lc * CH:(lc + 1) * CH]
                drain(rout, pin, 2 * CH)
                if lc == GROUPS[gidx] - 1:
                    assert pB is None
                    pB = pA
                    pA = {"rl": rl, "nch": GROUPS[gidx], "ocol": ocol,
                          "acc": None, "next_k": 0}
                    ocol += 2 * GROUPS[gidx] * GSEG
                    gstart = c + 1
                    gidx += 1
            # tail: finish whatever classifier work remains, oldest first
            for pend in (pB, pA):
                if pend is None:
                    continue
                while pend["next_k"] < 8:
                    pend_kslice(pend)
                mm2_finish(pend)

    nc.compile()
    return nc


def _pack_inputs(x, Wloc, W):
    x = np.asarray(x, dtype=np.float32)
    # [core, half, row, d] -> pad rows to 148*510 -> j-major per 510-chunk
    xp = x.reshape(N_CORES, 2, HALF, D_IN)
    pad = np.zeros((N_CORES, 2, COLS - HALF, D_IN), dtype=np.float32)
    xp = np.concatenate([xp, pad], axis=2)
    # within each chunk: source row = g*30 + j  ->  column j*17 + g
    xp = xp.reshape(N_CORES, 2, NCHUNK, GSEG, J, D_IN)
    xp = xp.transpose(0, 1, 2, 4, 3, 5)          # [., c, j, g, d]
    xp = xp.reshape(N_CORES, 2, COLS, D_IN)
    xp = xp.transpose(0, 1, 3, 2)                # [core, half, d, col]
    xp8 = np.ascontiguousarray(xp, dtype=np.float16).reshape(
        N_CORES, 128, COLS)

    w1 = np.ascontiguousarray(
        np.concatenate([Wloc.T, Wloc.T], axis=0), dtype=np.float16)
    w2 = np.ascontiguousarray((W / float(J)).T, dtype=np.float16)
    return xp8, w1, w2


def kernel(x: np.ndarray, Wloc: np.ndarray, W: np.ndarray) -> np.ndarray:
    if "nc" not in _CACHE:
        _CACHE["nc"] = _build_kernel()
    nc = _CACHE["nc"]

    xp8, w1, w2 = _pack_inputs(x, Wloc, W)
    in_maps = [{"xt": xp8[c], "w1": w1, "w2": w2} for c in range(N_CORES)]
    res = run_bass_kernel_spmd(nc, in_maps, core_ids=list(range(N_CORES)))
    _CACHE["exec_time_ns"] = res.exec_time_ns
    _CACHE["trace"] = res.instructions_and_trace

    out = np.empty((L // J, C), dtype=np.float32)
    for core in range(N_CORES):
        od = res.results[core]["out"]            # [128, OUT_COLS]
        oc = od[0:10] + od[32:42] + od[64:74] + od[96:106]  # [10, 5032]
        seg_vals = np.empty((2, SLOTS_H, C), dtype=np.float32)
        off = 0
        cstart = 0
        for nch in GROUPS:
            n = 2 * nch * GSEG
            blk = oc[:, off:off + n].reshape(C, 2, nch * GSEG)
            s0 = cstart * GSEG
            seg_vals[0, s0:s0 + nch * GSEG] = blk[:, 0].T
            seg_vals[1, s0:s0 + nch * GSEG] = blk[:, 1].T
            off += n
            cstart += nch
        base = core * (R // J)
        out[base:base + SEG_H] = seg_vals[0, :SEG_H]
        out[base + SEG_H:base + 2 * SEG_H] = seg_vals[1, :SEG_H]
    return out
